# revision 62
# baseline (speedup 1.0000x reference)
"""DeepFM forward kernel for Trainium2, data-parallel over 8 NeuronCores.

Math refactor vs the straightforward DeepFM graph:
  1. Tower dense outputs are never materialized: W1 is folded into the
     tower weights host-side (z1 = xm @ (Wm_d@W1[:256]) + xu @ (Wu_d@W1[256:])),
     and the FM interaction sum collapses to 16 fold sums per tower.
  2. The FM sum uses the polarization identity sum fold_m.fold_u + add =
     sum (p^2 - q^2)/4 + a (p/q linear in x; the additive term rides two
     rows as ((a+1)/2)^2 - ((a-1)/2)^2 = a): one matmul accumulation chain
     plus one scalar-engine Square, folded into the final matmul.

Precision scheme (fp8 DoubleRow everywhere the PE is hot):
  - Inputs ship as x8 = fp8(x) plus the scaled residual r8 = fp8(16*(x-x8)).
    Combined they carry ~12 mantissa bits at the same 4MB as one bf16 copy.
  - z1 matmuls: fp8 DoubleRow (K=256 per matmul, 2 multiplies/cell/cycle),
    weights scaled x16 into e4m3 range; the relu's scale=1/16 undoes it.
  - The precision-critical FM/extras chain runs three DoubleRow chains:
    F8^T x8 + (F8/16)^T r8 + Fr8^T x8  (weight-quantization residual Fr8),
    recovering bf16-level accuracy; the Square's scale=1/G undoes the
    weight scale G. MLP2/final stay bf16 on on-chip operands.
All matmuls run in uniform 128x128 PE tiling mode (narrow lhsT zero-padded
to M=128) so the array never pays a mode-switch drain.
"""

import numpy as np
import ml_dtypes

import concourse.bacc as bacc
import concourse.bass as bass  # noqa: F401
import concourse.mybir as mybir
import concourse.tile as tile
from concourse.bass_utils import run_bass_kernel_spmd

N_CORES = 8
B_FULL = 16384
R = B_FULL // N_CORES  # 2048 rows per core
F = 512                # input features per tower
KC = F // 128          # 4 contraction chunks per tower
NT = 512               # batch tile on the free dim
NTILES = R // NT       # 4
NX = 34                # extras rows: p(16) + q(16) + a-rows(2)
N_WARM = 23            # PE pre-warm matmuls (N=256): enough sustained PE
                       # busy to cross the ~3.4us HAM activity window BEFORE
                       # the chain ends (with margin for window phase) — once
                       # the clock gate opens, short idle gaps are harmless
XW = 128               # extras lhsT zero-padded to M=128 (uniform PE mode)
XT_COLS = 2 * KC * NT  # per-tile input cols (both towers)

F32 = mybir.dt.float32
BF16 = mybir.dt.bfloat16
E4M3 = mybir.dt.float8e4

Z1_COLS = 16 * 128     # fp8 z1 blob: [xm-g0 | xm-g1 | xu-g0 | xu-g1]
Z1_SCALE = 16.0
XG = 4.0               # extras weight scale (undone by the Square's scale)
RS = 16.0              # x-residual scale: r8 = fp8(RS * (x - x8))
NCH = 3                # extras chains: F8^T x8, (F8/RS)^T r8, Fr8^T x8

# bf16 weight-pack column offsets (W2 | W3 pad | WQ pad)
W2_OFF = 0
W3_OFF = 2 * 128
WQ_OFF = W3_OFF + 128
WCOLS = WQ_OFF + 128

# fp32 bias-pack column indices ([128, BCOLS])
B1A, B1B, BX, B2C = range(4)
BCOLS = 4


def _chunk3(Wext, kc=8):
    """[K, M] -> [128, kc, M]: chunk k = rows k*128..(k+1)*128."""
    m = Wext.shape[1]
    return np.ascontiguousarray(Wext.reshape(kc, 128, m).transpose(1, 0, 2))


def _col(vec):
    out = np.zeros((128, 1), np.float32)
    out[: len(vec), 0] = vec
    return out


def _pack_weights(Wm, bm, Wu, bu, W1, b1, W2, b2, W3, b3):
    f64 = np.float64
    fp8 = lambda a: np.asarray(a, ml_dtypes.float8_e4m3).astype(f64)
    Wm, bm, Wu, bu = Wm.astype(f64), bm.astype(f64), Wu.astype(f64), bu.astype(f64)
    W1, b1, W2, b2 = W1.astype(f64), b1.astype(f64), W2.astype(f64), b2.astype(f64)
    b3v = float(np.asarray(b3, f64).reshape(-1)[0])

    Am = Wm[:, :256] @ W1[:256, :]
    Au = Wu[:, :256] @ W1[256:, :]
    b1p = b1 + bm[:256] @ W1[:256, :] + bu[:256] @ W1[256:, :]

    FWm = Wm[:, :256].reshape(F, 16, 16).sum(axis=1)
    FWu = Wu[:, :256].reshape(F, 16, 16).sum(axis=1)
    fbm = bm[:256].reshape(16, 16).sum(axis=0)
    fbu = bu[:256].reshape(16, 16).sum(axis=0)
    awm, awu = Wm[:, 256], Wu[:, 256]
    A = bm[256] + bu[256] + b3v
    Xm = np.concatenate([FWm, FWm, awm[:, None] / 2, awm[:, None] / 2], axis=1)
    Xu = np.concatenate([FWu, -FWu, awu[:, None] / 2, awu[:, None] / 2], axis=1)
    xbias = np.concatenate([fbm + fbu, fbm - fbu, [(A + 1) / 2], [(A - 1) / 2]])
    wq = np.concatenate([np.full(16, 0.25), np.full(16, -0.25), [1.0, -1.0]])

    # fp8 z1 blob [128, 2(half), 2(g), KC, 128], scaled x16
    amc, auc = _chunk3(Am, KC), _chunk3(Au, KC)  # [128, 4, 256]
    w8 = np.stack(
        [
            np.stack([amc[:, :, :128], amc[:, :, 128:]], axis=1),
            np.stack([auc[:, :, :128], auc[:, :, 128:]], axis=1),
        ],
        axis=1,
    )  # [128, half, g, KC, 128]
    w8 = (w8 * Z1_SCALE).astype(ml_dtypes.float8_e4m3)

    # fp8 extras blob [128, NCH, 8, XW]: chains c1=F8, c2=F8/RS, c3=Fr8
    XWmat = np.concatenate([Xm, Xu], axis=0)          # [1024, 34]
    XWpad = np.zeros((2 * F, XW), f64)
    XWpad[:, :NX] = XWmat
    F8 = fp8(XWpad * XG)
    Fr = XWpad * XG - F8
    wx8 = np.stack(
        [_chunk3(F8), _chunk3(F8 / RS), _chunk3(Fr)], axis=1
    )  # [128, 3, 8, XW]
    wx8 = wx8.astype(ml_dtypes.float8_e4m3)

    w3_pad = np.zeros((128, 128), f64)
    w3_pad[:, 0] = np.asarray(W3, f64).reshape(128)
    wq_pad = np.zeros((128, 128), f64)
    wq_pad[:NX, 0] = wq
    w28 = (_chunk3(W2, 2) * Z1_SCALE).astype(ml_dtypes.float8_e4m3)
    wp = np.concatenate(
        [_chunk3(W2, 2).reshape(128, 256), w3_pad, wq_pad], axis=1
    )
    assert wp.shape == (128, WCOLS), wp.shape
    bp = np.concatenate(
        [_col(b1p[:128]), _col(b1p[128:]), _col(xbias), _col(b2)], axis=1
    )
    return (
        np.ascontiguousarray(w8.reshape(128, Z1_COLS)),
        np.ascontiguousarray(w28.reshape(128, 256)),
        np.ascontiguousarray(wx8.reshape(128, NCH * 8 * XW)),
        np.ascontiguousarray(wp.astype(ml_dtypes.bfloat16)),
        np.ascontiguousarray(bp.astype(np.float32)),
    )


def _build_bass():
    nc = bacc.Bacc()
    x8 = nc.dram_tensor("x8", [128, NTILES * XT_COLS], E4M3, kind="ExternalInput")
    r8 = nc.dram_tensor("r8", [128, NTILES * XT_COLS], E4M3, kind="ExternalInput")
    w8d = nc.dram_tensor("w8", [128, Z1_COLS], E4M3, kind="ExternalInput")
    wx8d = nc.dram_tensor("wx8", [128, NCH * 8 * XW], E4M3, kind="ExternalInput")
    w28d = nc.dram_tensor("w28", [128, 256], E4M3, kind="ExternalInput")
    wpd = nc.dram_tensor("wp", [128, WCOLS], BF16, kind="ExternalInput")
    bpd = nc.dram_tensor("bp", [128, BCOLS], F32, kind="ExternalInput")
    out = nc.dram_tensor("out", [1, R], F32, kind="ExternalOutput")

    relu = mybir.ActivationFunctionType.Relu
    square = mybir.ActivationFunctionType.Square
    DR = mybir.MatmulPerfMode.DoubleRow

    with tile.TileContext(nc) as tc:
        with (
            tc.tile_pool(name="wpool", bufs=1) as wpool,
            tc.tile_pool(name="xpool", bufs=1) as xpool,
            tc.tile_pool(name="dpool", bufs=1) as dpool,
            tc.tile_pool(name="opool", bufs=1) as opool,
            tc.tile_pool(name="psz", bufs=3, space="PSUM") as psz,
            tc.tile_pool(name="psx", bufs=2, space="PSUM") as psx,
            tc.tile_pool(name="psm", bufs=1, space="PSUM") as psm,
            tc.tile_pool(name="psf", bufs=2, space="PSUM") as psf,
        ):
            # PE pre-warm (see N_WARM note)
            wgar = wpool.tile([128, NT], BF16)
            nc.gpsimd.memset(wgar, 0.0)
            for _ in range(N_WARM):
                pw = psz.tile([128, NT], F32, name="ps_z1")
                nc.tensor.matmul(
                    pw[:, :256], wgar[:, :128], wgar[:, :256],
                    start=True, stop=True,
                )

            # weights on the scalar ring in consumption order
            H = Z1_COLS // 2
            w8m = wpool.tile([128, 2, KC, 128], E4M3)
            nc.scalar.dma_start(out=w8m, in_=w8d[:, :H])
            w8u = wpool.tile([128, 2, KC, 128], E4M3)
            nc.scalar.dma_start(out=w8u, in_=w8d[:, H:])
            b = wpool.tile([128, BCOLS], F32)
            nc.scalar.dma_start(out=b, in_=bpd[:, :])
            wrm = wpool.tile([128, WCOLS], BF16)
            nc.scalar.dma_start(out=wrm, in_=wpd[:, :])
            wx8 = wpool.tile([128, NCH, 8, XW], E4M3)
            nc.scalar.dma_start(out=wx8, in_=wx8d[:, :])
            w28 = wpool.tile([128, 2, 128], E4M3)
            nc.scalar.dma_start(out=w28, in_=w28d[:, :])
            out_sb = opool.tile([1, NTILES * NT], F32)

            x8r = x8.rearrange("p (t w c n) -> p t w c n", t=NTILES, w=2, c=KC, n=NT)
            r8r = r8.rearrange("p (t w c n) -> p t w c n", t=NTILES, w=2, c=KC, n=NT)

            # inputs on the sync ring: x8 then r8 per tile (consumption
            # order); tile-0 x8 split per tower for the earliest start
            x80ma = xpool.tile([128, 2, NT], E4M3)
            nc.sync.dma_start(out=x80ma, in_=x8r[:, 0, 0, :2])
            x80mb = xpool.tile([128, 2, NT], E4M3)
            nc.sync.dma_start(out=x80mb, in_=x8r[:, 0, 0, 2:])
            x80u = xpool.tile([128, KC, NT], E4M3)
            nc.sync.dma_start(out=x80u, in_=x8r[:, 0, 1])
            r80 = xpool.tile([128, 2, KC, NT], E4M3)
            nc.sync.dma_start(out=r80, in_=r8r[:, 0])
            class _X0m:
                def __getitem__(self, idx):
                    # idx = (slice(None), slice(2p, 2p+2), slice(None)):
                    # each chunk pair is exactly one piece tile
                    return (x80ma, x80mb)[idx[1].start // 2]

            x8ts = [(_X0m(), x80u)]
            r8ts = [(r80[:, 0], r80[:, 1])]
            for t in range(1, NTILES):
                x8t = xpool.tile([128, 2, KC, NT], E4M3, name=f"x8_{t}")
                nc.sync.dma_start(out=x8t, in_=x8r[:, t])
                x8ts.append((x8t[:, 0], x8t[:, 1]))
                r8t = xpool.tile([128, 2, KC, NT], E4M3, name=f"r8_{t}")
                nc.sync.dma_start(out=r8t, in_=r8r[:, t])
                r8ts.append((r8t[:, 0], r8t[:, 1]))

            h1s, sqs, h2s = {}, {}, {}

            def emit_z1(t):
                # fp8 DoubleRow: each matmul contracts K=256 (two chunks as
                # a [128, 2, *] AP). xm feeds both output groups first so
                # the xu input can land meanwhile.
                xm8, xu8 = x8ts[t]
                pss = []
                for g in range(2):
                    pss.append(psz.tile([128, NT], F32, name="ps_z1"))
                for half, xf in ((0, xm8), (1, xu8)):
                    w8t = (w8m, w8u)[half]
                    for g in range(2):
                        for p in range(KC // 2):
                            nc.tensor.matmul(
                                pss[g],
                                w8t[:, g, 2 * p : 2 * p + 2, :],
                                xf[:, 2 * p : 2 * p + 2, :],
                                start=(half == 0 and p == 0),
                                stop=(half == 1 and p == KC // 2 - 1),
                                perf_mode=DR,
                            )
                        if half == 1:
                            if t not in h1s:
                                h1s[t] = dpool.tile(
                                    [128, 2, NT], E4M3, name=f"h1_{t}"
                                )
                            nc.scalar.activation(
                                out=h1s[t][:, g, :], in_=pss[g], func=relu,
                                bias=b[:, g : g + 1], scale=1.0 / Z1_SCALE,
                            )

            def emit_extras(t):
                # three DoubleRow chains accumulate G * (XW^T x) in fp32:
                # F8^T x8 + (F8/RS)^T r8 + Fr8^T x8; the Square's
                # scale=1/G recovers the true pre-activation.
                ps = psx.tile([128, NT], F32, name="ps_x")
                first, last = (0, 0, 0), (NCH - 1, 1, KC // 2 - 1)
                for c in range(NCH):
                    src = r8ts[t] if c == 1 else x8ts[t]
                    for tw in range(2):
                        xf = src[tw]
                        for p in range(KC // 2):
                            nc.tensor.matmul(
                                ps,
                                wx8[:, c, tw * KC + 2 * p : tw * KC + 2 * p + 2, :],
                                xf[:, 2 * p : 2 * p + 2, :],
                                start=((c, tw, p) == first),
                                stop=((c, tw, p) == last),
                                perf_mode=DR,
                            )
                sq = dpool.tile([128, NT], BF16, name=f"sq_{t}")
                nc.scalar.activation(
                    out=sq, in_=ps, func=square,
                    bias=b[:, BX : BX + 1], scale=1.0 / XG,
                )
                sqs[t] = sq

            def emit_mlp2(t):
                # one fp8 DoubleRow matmul (K=256); W2 scaled x16, undone by
                # the relu's scale
                ps = psm.tile([128, NT], F32, name="ps_m")
                nc.tensor.matmul(
                    ps, w28[:, 0:2, :], h1s[t][:, 0:2, :],
                    start=True, stop=True, perf_mode=DR,
                )
                h2 = dpool.tile([128, NT], BF16, name=f"h2_{t}")
                nc.scalar.activation(
                    out=h2, in_=ps, func=relu,
                    bias=b[:, B2C : B2C + 1], scale=1.0 / Z1_SCALE,
                )
                h2s[t] = h2

            def emit_final(t):
                # sq matmul first: its operand is ready well before h2
                ps = psf.tile([128, NT], F32, name="ps_f")
                nc.tensor.matmul(
                    ps, wrm[:, WQ_OFF : WQ_OFF + 128], sqs[t],
                    start=True, stop=False,
                )
                nc.tensor.matmul(
                    ps, wrm[:, W3_OFF : W3_OFF + 128], h2s[t],
                    start=False, stop=True,
                )
                n0 = t * NT
                # per-tile staging columns: copy_t never WAR-serializes
                # against the previous tile's still-reading out DMA
                ob = out_sb[0:1, n0 : n0 + NT]
                nc.scalar.copy(ob, ps[0:1])
                nc.scalar.dma_start(out=out[:, n0 : n0 + NT], in_=ob)

            for t in range(NTILES):
                emit_z1(t)
                if t > 0:
                    emit_mlp2(t - 1)
                emit_extras(t)
                if t > 0:
                    emit_final(t - 1)
            emit_mlp2(NTILES - 1)
            emit_final(NTILES - 1)
    nc.finalize()
    return nc


def _pack_x(xmT_core, xuT_core):
    """2x [512, 2048] fp32 -> ([128, .] fp8 x8, [128, .] fp8 r8)."""
    ym = xmT_core.reshape(KC, 128, NTILES, NT).transpose(1, 2, 0, 3)
    yu = xuT_core.reshape(KC, 128, NTILES, NT).transpose(1, 2, 0, 3)
    y = np.stack([ym, yu], axis=2).reshape(128, NTILES * XT_COLS)
    x8 = y.astype(ml_dtypes.float8_e4m3)
    r8 = ((y - x8.astype(np.float32)) * RS).astype(ml_dtypes.float8_e4m3)
    return np.ascontiguousarray(x8), np.ascontiguousarray(r8)


_NC_CACHE = []


def kernel(movie_vectors, user_vectors, Wm, bm, Wu, bu, W1, b1, W2, b2, W3, b3):
    movie_vectors = np.asarray(movie_vectors, np.float32)
    user_vectors = np.asarray(user_vectors, np.float32)
    w8, w28, wx8, wp, bp = _pack_weights(
        np.asarray(Wm, np.float32), np.asarray(bm, np.float32),
        np.asarray(Wu, np.float32), np.asarray(bu, np.float32),
        np.asarray(W1, np.float32), np.asarray(b1, np.float32),
        np.asarray(W2, np.float32), np.asarray(b2, np.float32),
        np.asarray(W3, np.float32), np.asarray(b3, np.float32),
    )
    xmT = movie_vectors.T  # [512, 16384]
    xuT = user_vectors.T

    if not _NC_CACHE:
        _NC_CACHE.append(_build_bass())
    nc = _NC_CACHE[0]

    in_maps = []
    for c in range(N_CORES):
        sl = slice(c * R, (c + 1) * R)
        x8a, r8a = _pack_x(xmT[:, sl], xuT[:, sl])
        in_maps.append(
            {
                "x8": x8a, "r8": r8a, "w8": w8, "w28": w28,
                "wx8": wx8, "wp": wp, "bp": bp,
            }
        )
    res = run_bass_kernel_spmd(nc, in_maps, core_ids=list(range(N_CORES)))
    kernel.last_result = res
    return np.concatenate([r["out"].reshape(R, 1) for r in res.results], axis=0)


# revision 64
# speedup vs baseline: 1.0056x; 1.0056x over previous
"""DeepFM forward kernel for Trainium2, data-parallel over 8 NeuronCores.

Math refactor vs the straightforward DeepFM graph:
  1. Tower dense outputs are never materialized: W1 is folded into the
     tower weights host-side (z1 = xm @ (Wm_d@W1[:256]) + xu @ (Wu_d@W1[256:])),
     and the FM interaction sum collapses to 16 fold sums per tower.
  2. The FM sum uses the polarization identity sum fold_m.fold_u + add =
     sum (p^2 - q^2)/4 + a (p/q linear in x; the additive term rides two
     rows as ((a+1)/2)^2 - ((a-1)/2)^2 = a): one matmul accumulation chain
     plus one scalar-engine Square, folded into the final matmul.

Precision scheme (fp8 DoubleRow everywhere the PE is hot):
  - Inputs ship as x8 = fp8(x) plus the scaled residual r8 = fp8(16*(x-x8)).
    Combined they carry ~12 mantissa bits at the same 4MB as one bf16 copy.
  - z1 matmuls: fp8 DoubleRow (K=256 per matmul, 2 multiplies/cell/cycle),
    weights scaled x16 into e4m3 range; the relu's scale=1/16 undoes it.
  - The precision-critical FM/extras chain runs three DoubleRow chains:
    F8^T x8 + (F8/16)^T r8 + Fr8^T x8  (weight-quantization residual Fr8),
    recovering bf16-level accuracy; the Square's scale=1/G undoes the
    weight scale G. MLP2/final stay bf16 on on-chip operands.
All matmuls run in uniform 128x128 PE tiling mode (narrow lhsT zero-padded
to M=128) so the array never pays a mode-switch drain.
"""

import numpy as np
import ml_dtypes

import concourse.bacc as bacc
import concourse.bass as bass  # noqa: F401
import concourse.mybir as mybir
import concourse.tile as tile
from concourse.bass_utils import run_bass_kernel_spmd

N_CORES = 8
B_FULL = 16384
R = B_FULL // N_CORES  # 2048 rows per core
F = 512                # input features per tower
KC = F // 128          # 4 contraction chunks per tower
NT = 512               # batch tile on the free dim
NTILES = R // NT       # 4
NX = 34                # extras rows: p(16) + q(16) + a-rows(2)
N_WARM = 23            # PE pre-warm matmuls (N=256): enough sustained PE
                       # busy to cross the ~3.4us HAM activity window BEFORE
                       # the chain ends (with margin for window phase) — once
                       # the clock gate opens, short idle gaps are harmless
XW = 128               # extras lhsT zero-padded to M=128 (uniform PE mode)
XT_COLS = 2 * KC * NT  # per-tile input cols (both towers)

F32 = mybir.dt.float32
BF16 = mybir.dt.bfloat16
E4M3 = mybir.dt.float8e4

Z1_COLS = 16 * 128     # fp8 z1 blob: [xm-g0 | xm-g1 | xu-g0 | xu-g1]
Z1_SCALE = 16.0
XG = 4.0               # extras weight scale (undone by the Square's scale)
RS = 16.0              # x-residual scale: r8 = fp8(RS * (x - x8))
NCH = 3                # extras chains: F8^T x8, (F8/RS)^T r8, Fr8^T x8

# bf16 weight-pack column offsets (W2 | W3 pad | WQ pad)
W2_OFF = 0
W3_OFF = 2 * 128
WQ_OFF = W3_OFF + 128
WCOLS = WQ_OFF + 128

# fp32 bias-pack column indices ([128, BCOLS])
B1A, B1B, BX, B2C = range(4)
BCOLS = 4


def _chunk3(Wext, kc=8):
    """[K, M] -> [128, kc, M]: chunk k = rows k*128..(k+1)*128."""
    m = Wext.shape[1]
    return np.ascontiguousarray(Wext.reshape(kc, 128, m).transpose(1, 0, 2))


def _col(vec):
    out = np.zeros((128, 1), np.float32)
    out[: len(vec), 0] = vec
    return out


def _pack_weights(Wm, bm, Wu, bu, W1, b1, W2, b2, W3, b3):
    f64 = np.float64
    fp8 = lambda a: np.asarray(a, ml_dtypes.float8_e4m3).astype(f64)
    Wm, bm, Wu, bu = Wm.astype(f64), bm.astype(f64), Wu.astype(f64), bu.astype(f64)
    W1, b1, W2, b2 = W1.astype(f64), b1.astype(f64), W2.astype(f64), b2.astype(f64)
    b3v = float(np.asarray(b3, f64).reshape(-1)[0])

    Am = Wm[:, :256] @ W1[:256, :]
    Au = Wu[:, :256] @ W1[256:, :]
    b1p = b1 + bm[:256] @ W1[:256, :] + bu[:256] @ W1[256:, :]

    FWm = Wm[:, :256].reshape(F, 16, 16).sum(axis=1)
    FWu = Wu[:, :256].reshape(F, 16, 16).sum(axis=1)
    fbm = bm[:256].reshape(16, 16).sum(axis=0)
    fbu = bu[:256].reshape(16, 16).sum(axis=0)
    awm, awu = Wm[:, 256], Wu[:, 256]
    A = bm[256] + bu[256] + b3v
    Xm = np.concatenate([FWm, FWm, awm[:, None] / 2, awm[:, None] / 2], axis=1)
    Xu = np.concatenate([FWu, -FWu, awu[:, None] / 2, awu[:, None] / 2], axis=1)
    xbias = np.concatenate([fbm + fbu, fbm - fbu, [(A + 1) / 2], [(A - 1) / 2]])
    wq = np.concatenate([np.full(16, 0.25), np.full(16, -0.25), [1.0, -1.0]])

    # fp8 z1 blob [128, 2(half), 2(g), KC, 128], scaled x16
    amc, auc = _chunk3(Am, KC), _chunk3(Au, KC)  # [128, 4, 256]
    w8 = np.stack(
        [
            np.stack([amc[:, :, :128], amc[:, :, 128:]], axis=1),
            np.stack([auc[:, :, :128], auc[:, :, 128:]], axis=1),
        ],
        axis=1,
    )  # [128, half, g, KC, 128]
    w8 = (w8 * Z1_SCALE).astype(ml_dtypes.float8_e4m3)

    # fp8 extras blob [128, NCH, 8, XW]: chains c1=F8, c2=F8/RS, c3=Fr8
    XWmat = np.concatenate([Xm, Xu], axis=0)          # [1024, 34]
    XWpad = np.zeros((2 * F, XW), f64)
    XWpad[:, :NX] = XWmat
    F8 = fp8(XWpad * XG)
    Fr = XWpad * XG - F8
    F8c, Frc, F8rc = _chunk3(F8), _chunk3(Fr), _chunk3(F8 / RS)
    wx8 = np.stack(
        [F8c[:, 0:4], Frc[:, 0:4], F8c[:, 4:8], Frc[:, 4:8],
         F8rc[:, 0:4], F8rc[:, 4:8]], axis=1
    )  # [128, 6(group), 4(chunk), XW] in consumption order
    wx8 = wx8.astype(ml_dtypes.float8_e4m3)

    w3_pad = np.zeros((128, 128), f64)
    w3_pad[:, 0] = np.asarray(W3, f64).reshape(128)
    wq_pad = np.zeros((128, 128), f64)
    wq_pad[:NX, 0] = wq
    w28 = (_chunk3(W2, 2) * Z1_SCALE).astype(ml_dtypes.float8_e4m3)
    wp = np.concatenate(
        [_chunk3(W2, 2).reshape(128, 256), w3_pad, wq_pad], axis=1
    )
    assert wp.shape == (128, WCOLS), wp.shape
    bp = np.concatenate(
        [_col(b1p[:128]), _col(b1p[128:]), _col(xbias), _col(b2)], axis=1
    )
    return (
        np.ascontiguousarray(w8.reshape(128, Z1_COLS)),
        np.ascontiguousarray(w28.reshape(128, 256)),
        np.ascontiguousarray(wx8.reshape(128, NCH * 8 * XW)),
        np.ascontiguousarray(wp.astype(ml_dtypes.bfloat16)),
        np.ascontiguousarray(bp.astype(np.float32)),
    )


def _build_bass():
    nc = bacc.Bacc()
    x8 = nc.dram_tensor("x8", [128, NTILES * XT_COLS], E4M3, kind="ExternalInput")
    r8 = nc.dram_tensor("r8", [128, NTILES * XT_COLS], E4M3, kind="ExternalInput")
    w8d = nc.dram_tensor("w8", [128, Z1_COLS], E4M3, kind="ExternalInput")
    wx8d = nc.dram_tensor("wx8", [128, NCH * 8 * XW], E4M3, kind="ExternalInput")
    w28d = nc.dram_tensor("w28", [128, 256], E4M3, kind="ExternalInput")
    wpd = nc.dram_tensor("wp", [128, WCOLS], BF16, kind="ExternalInput")
    bpd = nc.dram_tensor("bp", [128, BCOLS], F32, kind="ExternalInput")
    out = nc.dram_tensor("out", [1, R], F32, kind="ExternalOutput")

    relu = mybir.ActivationFunctionType.Relu
    square = mybir.ActivationFunctionType.Square
    DR = mybir.MatmulPerfMode.DoubleRow

    with tile.TileContext(nc) as tc:
        with (
            tc.tile_pool(name="wpool", bufs=1) as wpool,
            tc.tile_pool(name="xpool", bufs=1) as xpool,
            tc.tile_pool(name="dpool", bufs=1) as dpool,
            tc.tile_pool(name="opool", bufs=1) as opool,
            tc.tile_pool(name="psz", bufs=3, space="PSUM") as psz,
            tc.tile_pool(name="psx", bufs=2, space="PSUM") as psx,
            tc.tile_pool(name="psm", bufs=1, space="PSUM") as psm,
            tc.tile_pool(name="psf", bufs=2, space="PSUM") as psf,
        ):
            # PE pre-warm (see N_WARM note)
            wgar = wpool.tile([128, NT], BF16)
            nc.gpsimd.memset(wgar, 0.0)
            for _ in range(N_WARM):
                pw = psz.tile([128, NT], F32, name="ps_z1")
                nc.tensor.matmul(
                    pw[:, :256], wgar[:, :128], wgar[:, :256],
                    start=True, stop=True,
                )

            # weights on the scalar ring in consumption order
            H = Z1_COLS // 2
            w8m = wpool.tile([128, 2, KC, 128], E4M3)
            nc.scalar.dma_start(out=w8m, in_=w8d[:, :H])
            w8u = wpool.tile([128, 2, KC, 128], E4M3)
            nc.scalar.dma_start(out=w8u, in_=w8d[:, H:])
            wx8a = wpool.tile([128, 2, 4, XW], E4M3)
            nc.scalar.dma_start(out=wx8a, in_=wx8d[:, : 2 * 4 * XW])
            b = wpool.tile([128, BCOLS], F32)
            nc.scalar.dma_start(out=b, in_=bpd[:, :])
            wrm = wpool.tile([128, WCOLS], BF16)
            nc.scalar.dma_start(out=wrm, in_=wpd[:, :])
            w28 = wpool.tile([128, 2, 128], E4M3)
            nc.scalar.dma_start(out=w28, in_=w28d[:, :])
            wx8b = wpool.tile([128, 4, 4, XW], E4M3)
            nc.scalar.dma_start(out=wx8b, in_=wx8d[:, 2 * 4 * XW :])
            out_sb = opool.tile([1, NTILES * NT], F32)

            x8r = x8.rearrange("p (t w c n) -> p t w c n", t=NTILES, w=2, c=KC, n=NT)
            r8r = r8.rearrange("p (t w c n) -> p t w c n", t=NTILES, w=2, c=KC, n=NT)

            # inputs on the sync ring: x8 then r8 per tile (consumption
            # order); tile-0 x8 split per tower for the earliest start
            x80m = xpool.tile([128, KC, NT], E4M3)
            nc.sync.dma_start(out=x80m, in_=x8r[:, 0, 0])
            x80u = xpool.tile([128, KC, NT], E4M3)
            nc.sync.dma_start(out=x80u, in_=x8r[:, 0, 1])
            r80 = xpool.tile([128, 2, KC, NT], E4M3)
            nc.sync.dma_start(out=r80, in_=r8r[:, 0])
            x8ts = [(x80m, x80u)]
            r8ts = [(r80[:, 0], r80[:, 1])]
            for t in range(1, NTILES):
                x8t = xpool.tile([128, 2, KC, NT], E4M3, name=f"x8_{t}")
                nc.sync.dma_start(out=x8t, in_=x8r[:, t])
                x8ts.append((x8t[:, 0], x8t[:, 1]))
                r8t = xpool.tile([128, 2, KC, NT], E4M3, name=f"r8_{t}")
                nc.sync.dma_start(out=r8t, in_=r8r[:, t])
                r8ts.append((r8t[:, 0], r8t[:, 1]))

            h1s, sqs, h2s = {}, {}, {}

            def xgrp(g):
                # extras weight group in consumption order
                return wx8a[:, g] if g < 2 else wx8b[:, g - 2]

            def emit_main(t, mid=None):
                # z1 + extras interleaved, all-xm work first (z1-xm, then
                # extras c1/c3-xm): twice the matmuls covering the xu input
                # arrival. r8 chains (c2) run last, when r8 has landed.
                xm8, xu8 = x8ts[t]
                pss = []
                for g in range(2):
                    pss.append(psz.tile([128, NT], F32, name="ps_z1"))
                px = psx.tile([128, NT], F32, name="ps_x")
                for half, xf in ((0, xm8), (1, xu8)):
                    w8t = (w8m, w8u)[half]
                    for g in range(2):
                        for p in range(KC // 2):
                            nc.tensor.matmul(
                                pss[g],
                                w8t[:, g, 2 * p : 2 * p + 2, :],
                                xf[:, 2 * p : 2 * p + 2, :],
                                start=(half == 0 and p == 0),
                                stop=(half == 1 and p == KC // 2 - 1),
                                perf_mode=DR,
                            )
                        if half == 1:
                            if t not in h1s:
                                h1s[t] = dpool.tile(
                                    [128, 2, NT], E4M3, name=f"h1_{t}"
                                )
                            nc.scalar.activation(
                                out=h1s[t][:, g, :], in_=pss[g], func=relu,
                                bias=b[:, g : g + 1], scale=1.0 / Z1_SCALE,
                            )
                    for gi in (2 * half, 2 * half + 1):  # c1, c3 this half
                        for p in range(KC // 2):
                            nc.tensor.matmul(
                                px, xgrp(gi)[:, 2 * p : 2 * p + 2, :],
                                xf[:, 2 * p : 2 * p + 2, :],
                                start=(gi == 0 and p == 0), stop=False,
                                perf_mode=DR,
                            )
                if mid is not None:
                    mid()
                for half in range(2):  # c2: the r8 correction chains
                    rf = r8ts[t][half]
                    for p in range(KC // 2):
                        nc.tensor.matmul(
                            px, xgrp(4 + half)[:, 2 * p : 2 * p + 2, :],
                            rf[:, 2 * p : 2 * p + 2, :],
                            start=False,
                            stop=(half == 1 and p == KC // 2 - 1),
                            perf_mode=DR,
                        )
                sq = dpool.tile([128, NT], BF16, name=f"sq_{t}")
                nc.scalar.activation(
                    out=sq, in_=px, func=square,
                    bias=b[:, BX : BX + 1], scale=1.0 / XG,
                )
                sqs[t] = sq

            def emit_mlp2(t):
                # one fp8 DoubleRow matmul (K=256); W2 scaled x16, undone by
                # the relu's scale
                ps = psm.tile([128, NT], F32, name="ps_m")
                nc.tensor.matmul(
                    ps, w28[:, 0:2, :], h1s[t][:, 0:2, :],
                    start=True, stop=True, perf_mode=DR,
                )
                h2 = dpool.tile([128, NT], BF16, name=f"h2_{t}")
                nc.scalar.activation(
                    out=h2, in_=ps, func=relu,
                    bias=b[:, B2C : B2C + 1], scale=1.0 / Z1_SCALE,
                )
                h2s[t] = h2

            def emit_final(t):
                # sq matmul first: its operand is ready well before h2
                ps = psf.tile([128, NT], F32, name="ps_f")
                nc.tensor.matmul(
                    ps, wrm[:, WQ_OFF : WQ_OFF + 128], sqs[t],
                    start=True, stop=False,
                )
                nc.tensor.matmul(
                    ps, wrm[:, W3_OFF : W3_OFF + 128], h2s[t],
                    start=False, stop=True,
                )
                n0 = t * NT
                # per-tile staging columns: copy_t never WAR-serializes
                # against the previous tile's still-reading out DMA
                ob = out_sb[0:1, n0 : n0 + NT]
                nc.scalar.copy(ob, ps[0:1])
                nc.sync.dma_start(out=out[:, n0 : n0 + NT], in_=ob)

            for t in range(NTILES):
                mid = (lambda tt=t: emit_mlp2(tt - 1)) if t > 0 else None
                emit_main(t, mid=mid)
                if t > 0:
                    emit_final(t - 1)
            emit_mlp2(NTILES - 1)
            emit_final(NTILES - 1)
    nc.finalize()
    return nc


def _pack_x(xmT_core, xuT_core):
    """2x [512, 2048] fp32 -> ([128, .] fp8 x8, [128, .] fp8 r8)."""
    ym = xmT_core.reshape(KC, 128, NTILES, NT).transpose(1, 2, 0, 3)
    yu = xuT_core.reshape(KC, 128, NTILES, NT).transpose(1, 2, 0, 3)
    y = np.stack([ym, yu], axis=2).reshape(128, NTILES * XT_COLS)
    x8 = y.astype(ml_dtypes.float8_e4m3)
    r8 = ((y - x8.astype(np.float32)) * RS).astype(ml_dtypes.float8_e4m3)
    return np.ascontiguousarray(x8), np.ascontiguousarray(r8)


_NC_CACHE = []


def kernel(movie_vectors, user_vectors, Wm, bm, Wu, bu, W1, b1, W2, b2, W3, b3):
    movie_vectors = np.asarray(movie_vectors, np.float32)
    user_vectors = np.asarray(user_vectors, np.float32)
    w8, w28, wx8, wp, bp = _pack_weights(
        np.asarray(Wm, np.float32), np.asarray(bm, np.float32),
        np.asarray(Wu, np.float32), np.asarray(bu, np.float32),
        np.asarray(W1, np.float32), np.asarray(b1, np.float32),
        np.asarray(W2, np.float32), np.asarray(b2, np.float32),
        np.asarray(W3, np.float32), np.asarray(b3, np.float32),
    )
    xmT = movie_vectors.T  # [512, 16384]
    xuT = user_vectors.T

    if not _NC_CACHE:
        _NC_CACHE.append(_build_bass())
    nc = _NC_CACHE[0]

    in_maps = []
    for c in range(N_CORES):
        sl = slice(c * R, (c + 1) * R)
        x8a, r8a = _pack_x(xmT[:, sl], xuT[:, sl])
        in_maps.append(
            {
                "x8": x8a, "r8": r8a, "w8": w8, "w28": w28,
                "wx8": wx8, "wp": wp, "bp": bp,
            }
        )
    res = run_bass_kernel_spmd(nc, in_maps, core_ids=list(range(N_CORES)))
    kernel.last_result = res
    return np.concatenate([r["out"].reshape(R, 1) for r in res.results], axis=0)
